# revision 45
# baseline (speedup 1.0000x reference)
"""Trainium2 Bass kernel for GBMS mean-shift step (nn_GBMS_RNN_137438953906).

Math (per batch b):
    W = exp((X X^T - 1) / bandwidth^2)          [N, N]
    Y = (W @ X) / rowsum(W)                     [N, D]
    out = Y / max(||Y||_2, 1e-12)  (L2 norm along D)

Key identity: rowsum(W) is a positive per-row scalar, so it cancels in the
final L2 normalization:  l2norm(W@X / d) == l2norm(W@X).  We therefore never
compute row sums.

Sharding: data-parallel over batch B=8 across the 8 NeuronCores (1 batch
each).  Within a core, flash-attention-style row blocking: W is produced in
[128, 512] PSUM tiles and consumed immediately; the full NxN matrix is never
materialized.

Per-core dataflow:
  XT[d, n] = X^T / |bandwidth|      (PE transposes; scale folded in so the
                                     S matmul directly yields X X^T / b^2)
  for each group g of 512 output rows:
      for jb in 32 blocks of 128:
          S[j, n512] = XT[:,jb128]^T @ XT[:,g512]          (fp32r matmul)
          W[j, n512] = exp(S - 1/b^2)                      (ACT, 3-bank batches)
          YT[d, n512] += Xnat[:,jb,:]^T @ W                (fp32r matmul, PSUM acc)
      transpose YT -> Y[n, d] tiles, accumulate sum-of-squares
  1/norm = fast-inverse-sqrt(ss) + 2 Newton steps (DVE-only, no ACT tables)
  out = Y * (1/norm)
"""

import sys

if "/opt/trn_rl_repo" not in sys.path:
    sys.path.insert(0, "/opt/trn_rl_repo")

import numpy as np

import concourse.mybir as mybir
from concourse import bacc
from concourse.tile import TileContext
from concourse.bass_utils import run_bass_kernel_spmd
from concourse.masks import make_identity

P = 128
N = 4096
D = 128
NB = N // P  # 32 row blocks
G = N // 512  # 8 column groups
JT = 3  # j-blocks per exp batch (3 PSUM banks)
NCHUNK = 8  # input DMA chunks (4 row-blocks each)

F32 = mybir.dt.float32
F32R = mybir.dt.float32r

S_MM_DT = F32R
Y_MM_DT = F32R

_CACHED_NC = None


def _build():
    nc = bacc.Bacc("TRN2", target_bir_lowering=False, debug=False)

    x_in = nc.dram_tensor("X", [N, D], F32, kind="ExternalInput")
    bw_in = nc.dram_tensor("bandwidth", [1], F32, kind="ExternalInput")
    y_out = nc.dram_tensor("Y", [N, D], F32, kind="ExternalOutput")

    x_src = x_in.rearrange("(jb p) d -> p jb d", p=P)  # [128, 32, 128] view
    y_dst = y_out.rearrange("(nb p) d -> p nb d", p=P)

    with TileContext(nc) as tc:
        with (
            tc.tile_pool(name="const", bufs=1) as const,
            tc.tile_pool(name="spsum", bufs=2, space="PSUM") as s_pool,
            tc.tile_pool(name="ytpsum", bufs=1, space="PSUM") as yt_pool,
            tc.tile_pool(name="tppsum", bufs=1, space="PSUM") as tp_pool,
            tc.tile_pool(name="wpool", bufs=4) as w_pool,
            tc.tile_pool(name="stgpool", bufs=2) as stg_pool,
            tc.tile_pool(name="sqpool", bufs=2) as sq_pool,
        ):
            # ---- input chunk 0 first: it gates the whole pipeline and DMA
            # dispatches serialize on the SP sequencer (~500ns each) ----
            x_nat = const.tile([P, NB, D], F32)  # [j_in_block, jb, d]
            cb = NB // NCHUNK  # row blocks per chunk
            nc.sync.dma_start(x_nat[:, 0:cb, :], x_src[:, 0:cb, :])

            bw = const.tile([P, 1], F32)
            nc.sync.dma_start(bw[:], bw_in[None, :].to_broadcast([P, 1]))

            # identity built on the otherwise-idle GPSIMD engine (no DMA slot)
            ident = const.tile([P, P], F32)
            make_identity(nc, ident[:])

            for c in range(1, NCHUNK):
                nc.sync.dma_start(
                    x_nat[:, c * cb : (c + 1) * cb, :],
                    x_src[:, c * cb : (c + 1) * cb, :],
                )

            scr = const.tile([P, 5], F32)
            negb = scr[:, 0:1]
            absb = scr[:, 1:2]
            rb = scr[:, 2:3]
            negc = scr[:, 3:4]
            dummy = scr[:, 4:5]
            nc.vector.tensor_scalar_mul(negb, bw[:], -1.0)
            nc.vector.tensor_tensor(absb, bw[:], negb, mybir.AluOpType.max)
            nc.vector.reciprocal(rb, absb)  # 1/|b|
            nc.vector.tensor_tensor(negc, rb, rb, mybir.AluOpType.mult)
            nc.vector.tensor_scalar_mul(negc, negc, -1.0)  # -1/b^2

            # Preload the exp ACT table set while DMAs stream in -- the only
            # table load in the kernel (normalization is DVE-only).
            nc.scalar.activation(dummy, absb, mybir.ActivationFunctionType.Exp)

            # Junk transposes to start ramping the PE clock (HAM) while the
            # first input chunk is still in flight.
            warm_ps = s_pool.tile([P, JT, 512], F32, tag="s")
            for t in range(6):
                nc.tensor.transpose(
                    warm_ps[:, t // 4, (t % 4) * P : (t % 4 + 1) * P],
                    ident[:],
                    ident[:],
                )

            # fp32r copy of x_nat for the Y matmul (fp32r matmul operands
            # must be written pre-rounded by their producer)
            x_natr = const.tile([P, NB, D], Y_MM_DT)
            # XT = X^T / |b|, built per chunk via PE transposes
            xt = const.tile([P, N], S_MM_DT)

            chunks_done = [0]

            def emit_chunk(c, per_block=False):
                # 4 PE transposes -> one PSUM bank -> scaled copy to xt.
                # per_block pipelines transpose/copy at row-block granularity
                # (used for chunk 0, which gates the very first exp).
                if not per_block:
                    nc.vector.tensor_copy(
                        x_natr[:, c * cb : (c + 1) * cb, :],
                        x_nat[:, c * cb : (c + 1) * cb, :],
                    )
                t_ps = s_pool.tile([P, JT, 512], F32, tag="s")
                for o in range(cb):
                    jb = c * cb + o
                    nc.tensor.transpose(
                        t_ps[:, 0, o * P : (o + 1) * P], x_nat[:, jb, :], ident[:]
                    )
                    if per_block:
                        nc.vector.tensor_scalar_mul(
                            xt[:, jb * P : (jb + 1) * P],
                            t_ps[:, 0, o * P : (o + 1) * P],
                            rb,
                        )
                if not per_block:
                    nc.vector.tensor_scalar_mul(
                        xt[:, c * cb * P : (c + 1) * cb * P], t_ps[:, 0, :], rb
                    )
                else:
                    # x_natr is only needed by the (later) Y matmuls -- keep
                    # it off the startup critical path
                    nc.vector.tensor_copy(
                        x_natr[:, c * cb : (c + 1) * cb, :],
                        x_nat[:, c * cb : (c + 1) * cb, :],
                    )

            def need_chunks(upto):
                while chunks_done[0] <= min(upto, NCHUNK - 1):
                    emit_chunk(chunks_done[0], per_block=(chunks_done[0] == 0))
                    chunks_done[0] += 1

            # ---- output staging ----
            y_all = const.tile([P, NB, D], F32)  # [n_in_block, nb, d]
            ss_all = const.tile([P, NB], F32)
            half = const.tile([P, NB], F32)
            tmp = const.tile([P, NB], F32)
            rcp = const.tile([P, NB], F32)
            I32 = mybir.dt.int32
            magic = const.tile([P, NB], I32)
            shreg = const.tile([P, NB], I32)
            nc.vector.memset(magic[:], 0x5F3759DF)

            def normalize_and_store(g0, g1):
                """L2-normalize output rows of groups [g0, g1) and DMA out.

                1/norm = rsqrt(ss) via the fast-inverse-sqrt bit trick plus
                2 Newton iterations -- DVE-only, no ACT table switches, and
                ~4e-6 relative accuracy.  ss == 0 rows stay finite (y == 0
                there, matching the reference's eps-guarded division).
                """
                lo, hi = g0 * 4, g1 * 4  # nb range
                ss = ss_all[:, lo:hi]
                rs = rcp[:, lo:hi]
                hf = half[:, lo:hi]
                tm = tmp[:, lo:hi]
                nc.vector.tensor_scalar(
                    shreg[:, lo:hi],
                    ss.bitcast(I32),
                    1,
                    None,
                    mybir.AluOpType.logical_shift_right,
                )
                nc.vector.tensor_tensor(
                    rs.bitcast(I32),
                    magic[:, lo:hi],
                    shreg[:, lo:hi],
                    mybir.AluOpType.subtract,
                )
                nc.vector.tensor_scalar_mul(hf, ss, 0.5)
                for _ in range(2):
                    nc.vector.tensor_tensor(tm, rs, rs, mybir.AluOpType.mult)
                    nc.vector.tensor_tensor(tm, tm, hf, mybir.AluOpType.mult)
                    nc.vector.tensor_scalar(
                        tm, tm, -1.0, 1.5, mybir.AluOpType.mult, mybir.AluOpType.add
                    )
                    nc.vector.tensor_tensor(rs, rs, tm, mybir.AluOpType.mult)
                for nb in range(lo, hi):
                    nc.vector.tensor_scalar_mul(
                        y_all[:, nb, :], y_all[:, nb, :], rcp[:, nb : nb + 1]
                    )
                # split the store across DMA queues
                mid = (lo + hi) // 2
                nc.sync.dma_start(y_dst[:, lo:mid, :], y_all[:, lo:mid, :])
                if mid < hi:
                    nc.sync.dma_start(y_dst[:, mid:hi, :], y_all[:, mid:hi, :])

            def make_tail(g, yt):
                """Tail of group g: YT[d, n512] -> Y[n, d] + sum of squares.
                Emitted 2 triples into the NEXT group so the 4 PE transposes
                hide behind that group's ACT work instead of stalling it."""

                def tail():
                    stg = stg_pool.tile([P, 512], F32, tag="stg")
                    nc.vector.tensor_copy(stg[:], yt[:])
                    tp = tp_pool.tile([P, 4, P], F32, tag="tp")
                    for t in range(4):
                        nc.tensor.transpose(
                            tp[:, t, :], stg[:, t * P : (t + 1) * P], ident[:]
                        )
                    y_slice = y_all[:, g * 4 : (g + 1) * 4, :]
                    nc.vector.tensor_copy(y_slice, tp[:])
                    sq = sq_pool.tile([P, 4, P], F32, tag="sq")
                    nc.vector.tensor_tensor(
                        sq[:], y_slice, y_slice, mybir.AluOpType.mult
                    )
                    nc.vector.tensor_reduce(
                        ss_all[:, g * 4 : (g + 1) * 4],
                        sq[:],
                        axis=mybir.AxisListType.X,
                        op=mybir.AluOpType.add,
                    )

                return tail

            def emit_y(py):
                pjb, ptsz, pw, pyt = py
                for q in range(ptsz):
                    nc.tensor.matmul(
                        pyt[:],
                        x_natr[:, pjb + q, :],
                        pw[:, q, :],
                        start=(pjb + q == 0),
                        stop=(pjb + q == NB - 1),
                    )

            # ---- main flash loop ----
            pending_tail = None
            pending_y = []
            for g in range(G):
                yt = yt_pool.tile([P, 512], F32, tag="yt")
                n_lo = g * 512
                if g == 0:
                    need_chunks(0)  # rhs columns for group 0

                # group 0 ramps with small first batches so ACT starts ASAP
                # (its S tiles appear late: DMA chunk -> transpose -> copy)
                if g == 0:
                    sizes = [1, 2] + [JT] * 9 + [2]
                else:
                    sizes = [JT] * 10 + [2]
                assert sum(sizes) == NB
                jb = 0
                jt_idx = 0
                for tsz in sizes:
                    if g == 0:
                        # +1 lookahead: transpose chunk c+1 while the j-walk
                        # is still inside chunk c, hiding the xt copy latency.
                        # No lookahead before the first batch -- it would sit
                        # on the first-exp critical path.
                        ahead = 1 if jt_idx > 0 else 0
                        need_chunks((jb + tsz - 1) // cb + ahead)
                    s_t = s_pool.tile([P, JT, 512], F32, tag="s")
                    for q in range(tsz):
                        nc.tensor.matmul(
                            s_t[:, q, :],
                            xt[:, (jb + q) * P : (jb + q + 1) * P],
                            xt[:, n_lo : n_lo + 512],
                            start=True,
                            stop=True,
                        )
                    w_t = w_pool.tile([P, JT, 512], Y_MM_DT, tag="w")
                    nc.scalar.activation(
                        w_t[:, :tsz, :],
                        s_t[:, :tsz, :],
                        mybir.ActivationFunctionType.Exp,
                        bias=negc,
                        scale=1.0,
                    )
                    # Y matmuls run two exp-batches behind the S matmuls so PE
                    # always has independent S work queued when a group ends.
                    pending_y.append((jb, tsz, w_t, yt))
                    if len(pending_y) > 2:
                        emit_y(pending_y.pop(0))
                    jb += tsz
                    jt_idx += 1
                    if jt_idx == 2:
                        if pending_tail is not None:
                            pending_tail()
                            pending_tail = None
                        if g == G - 1:
                            # normalize finished groups while the last group
                            # is still computing (DVE-only, so this doesn't
                            # touch the busy ACT engine)
                            normalize_and_store(0, G - 1)

                pending_tail = make_tail(g, yt)

            for py in pending_y:
                emit_y(py)
            pending_tail()
            normalize_and_store(G - 1, G)

    nc.compile()
    return nc


def _get_nc():
    global _CACHED_NC
    if _CACHED_NC is None:
        _CACHED_NC = _build()
    return _CACHED_NC


def kernel(X: np.ndarray, bandwidth: np.ndarray, **run_kwargs):
    """Full-input entry point: X [8, 4096, 128] f32, bandwidth scalar f32.

    Returns [8, 4096, 128] f32. Distributes one batch per NeuronCore.
    """
    X = np.ascontiguousarray(X, dtype=np.float32)
    B = X.shape[0]
    assert X.shape == (B, N, D), X.shape
    bw = np.asarray(bandwidth, dtype=np.float32).reshape(1)

    nc = _get_nc()
    in_maps = [{"X": X[b], "bandwidth": bw} for b in range(B)]
    try:
        res = run_bass_kernel_spmd(nc, in_maps, core_ids=list(range(B)), **run_kwargs)
    except Exception:
        # The first execution after other jax-on-neuron work occasionally hits
        # a transient NRT_EXEC_UNIT_UNRECOVERABLE; a retry succeeds.
        res = run_bass_kernel_spmd(nc, in_maps, core_ids=list(range(B)), **run_kwargs)
    out = np.stack([res.results[b]["Y"] for b in range(B)], axis=0)
    kernel.last_results = res
    return out


if __name__ == "__main__":
    rng = np.random.default_rng(0)
    X = rng.standard_normal((8, N, D), dtype=np.float32)
    X /= np.linalg.norm(X, axis=-1, keepdims=True)
    out = kernel(X=X, bandwidth=np.float32(0.1))
    print("out shape", out.shape, "finite", np.isfinite(out).all())
